# revision 1
# baseline (speedup 1.0000x reference)
"""Trainium2 Bass kernel for a GNN node-aggregator.

Math (reference):
    out[n] = sum_k Linear(concat(v[n], u[k, n]))          with W = [Wv | Wu]
           = (sum_k u[k]) @ Wu.T  +  K * (v @ Wv.T)  +  K * b

The sum over neighbors commutes with the linear layer, so the kernel
streams the big [K, N, D] neighbors tensor once (memory bound),
accumulates the K-sum on the Vector engine, transposes 128x128 node
blocks on the Tensor engine (identity matmul), and finishes with two
small matmuls against host-preprocessed weights plus a bias add.

Distribution: nodes are sharded across 8 NeuronCores.  Every core runs
the same program over 6272 = 49*128 nodes; the core slices overlap
slightly (50000 is not divisible by 8*128) and the host gather keeps
each core's owned rows only.
"""

import numpy as np

N_NODES = 50000
K_NB = 32
D = 128  # in features
O = 128  # out features
P = 128  # SBUF partitions

N_CORES = 8
QB = 49                # 128-node blocks per core
NC_NODES = P * QB      # 6272 nodes per core (overlapped shard)
CHUNK_Q = 7            # q-blocks per pipelined chunk
N_CHUNKS = QB // CHUNK_Q


def _core_starts():
    step = N_NODES // N_CORES
    return [min(c * step, N_NODES - NC_NODES) for c in range(N_CORES)]


def _build(
    k_nb=K_NB,
    qb=QB,
    chunk_q=CHUNK_Q,
    repeats=1,
    k_bufs=6,
    dual_ring=False,
    explicit_copies=False,
    copies_on="any",  # "any" | "split" | "dve"
    slab_bufs=2,
):
    if explicit_copies:
        copies_on = "split"
    """Build the per-core Bass program (SPMD: same NEFF on all cores)."""
    import concourse.mybir as mybir
    import concourse.tile as tile
    from concourse import bacc

    f32 = mybir.dt.float32
    nc_nodes = P * qb
    n_chunks = qb // chunk_q
    assert qb % chunk_q == 0
    cw = chunk_q * D  # chunk width in free elements

    nc = bacc.Bacc(trn_type="TRN2", name="node_aggregator")
    nbr = nc.dram_tensor("nbr", [k_nb, nc_nodes, D], f32, kind="ExternalInput")
    vin = nc.dram_tensor("vin", [nc_nodes, D], f32, kind="ExternalInput")
    wut = nc.dram_tensor("wut", [D, O], f32, kind="ExternalInput")    # Wu.T
    wvtk = nc.dram_tensor("wvtk", [D, O], f32, kind="ExternalInput")  # K * Wv.T
    bbc = nc.dram_tensor("bbc", [P, O], f32, kind="ExternalInput")    # K*b rows
    iden = nc.dram_tensor("iden", [P, P], f32, kind="ExternalInput")
    out = nc.dram_tensor("out", [nc_nodes, O], f32, kind="ExternalOutput")

    # Partition p holds nodes [qb*p, qb*p + qb): contiguous 49*512B per
    # partition in DRAM, so every chunk DMA is 128 x 3.5KB contiguous runs.
    nbr_r = nbr[:].rearrange("k (p q) d -> k p (q d)", p=P)
    v_r = vin[:].rearrange("(p q) d -> p (q d)", p=P)
    out_r = out[:].rearrange("(p q) o -> p (q o)", p=P)

    with tile.TileContext(nc) as tc:
        with (
            tc.tile_pool(name="cpool", bufs=1) as cpool,
            tc.tile_pool(name="kpool", bufs=k_bufs) as kpool,
            tc.tile_pool(name="apool", bufs=slab_bufs) as apool,
            tc.tile_pool(name="vpool", bufs=slab_bufs) as vpool,
            tc.tile_pool(name="opool", bufs=slab_bufs) as opool,
            tc.tile_pool(name="bpool", bufs=3) as bpool,
            tc.tile_pool(name="ptp", bufs=2, space="PSUM") as ptp,
            tc.tile_pool(name="pop", bufs=2, space="PSUM") as pop,
        ):
            wut_t = cpool.tile([D, O], f32)
            nc.sync.dma_start(wut_t[:], wut[:])
            wvtk_t = cpool.tile([D, O], f32)
            nc.sync.dma_start(wvtk_t[:], wvtk[:])
            bbc_t = cpool.tile([P, O], f32)
            nc.sync.dma_start(bbc_t[:], bbc[:])
            iden_t = cpool.tile([P, P], f32)
            nc.sync.dma_start(iden_t[:], iden[:])

            for _ in range(repeats):
                for c in range(n_chunks):
                    cs = slice(c * cw, (c + 1) * cw)
                    # K-sum of this chunk's neighbor slabs, in place on S.
                    S = apool.tile([P, cw], f32, tag="S")
                    nc.sync.dma_start(S[:], nbr_r[0, :, cs])
                    for k in range(1, k_nb):
                        kt = kpool.tile([P, cw], f32, tag="kt")
                        dma_eng = nc.scalar if (dual_ring and k % 2) else nc.sync
                        dma_eng.dma_start(kt[:], nbr_r[k, :, cs])
                        nc.vector.tensor_add(out=S[:], in0=S[:], in1=kt[:])
                    vt = vpool.tile([P, cw], f32, tag="vt")
                    nc.sync.dma_start(vt[:], v_r[:, cs])
                    ot = opool.tile([P, cw], f32, tag="ot")
                    for qq in range(chunk_q):
                        qs = slice(qq * D, (qq + 1) * D)
                        # PE transpose S block and v block to [d, n] layout.
                        pt1 = ptp.tile([D, P], f32, tag="pt1")
                        nc.tensor.transpose(pt1[:], S[:, qs], iden_t[:])
                        st = bpool.tile([D, P], f32, tag="st")
                        if copies_on == "split":
                            nc.scalar.copy(st[:], pt1[:])
                        elif copies_on == "dve":
                            nc.vector.tensor_copy(out=st[:], in_=pt1[:])
                        else:
                            nc.any.tensor_copy(out=st[:], in_=pt1[:])
                        pt2 = ptp.tile([D, P], f32, tag="pt2")
                        nc.tensor.transpose(pt2[:], vt[:, qs], iden_t[:])
                        vq = bpool.tile([D, P], f32, tag="vq")
                        if copies_on in ("split", "dve"):
                            nc.vector.tensor_copy(out=vq[:], in_=pt2[:])
                        else:
                            nc.any.tensor_copy(out=vq[:], in_=pt2[:])
                        # out_block = S_blk @ Wu.T + v_blk @ (K Wv).T (+ K b)
                        op = pop.tile([P, O], f32, tag="op")
                        nc.tensor.matmul(
                            op[:], lhsT=st[:], rhs=wut_t[:], start=True, stop=False
                        )
                        nc.tensor.matmul(
                            op[:], lhsT=vq[:], rhs=wvtk_t[:], start=False, stop=True
                        )
                        nc.vector.tensor_add(out=ot[:, qs], in0=op[:], in1=bbc_t[:])
                    nc.sync.dma_start(out_r[:, cs], ot[:])
    nc.compile()
    return nc


def _prep_weights(W, b):
    Wv = W[:, :D]
    Wu = W[:, D:]
    wut = np.ascontiguousarray(Wu.T, dtype=np.float32)
    wvtk = np.ascontiguousarray(Wv.T * np.float32(K_NB), dtype=np.float32)
    bbc = np.ascontiguousarray(
        np.broadcast_to((np.float32(K_NB) * b).astype(np.float32), (P, O))
    )
    iden = np.eye(P, dtype=np.float32)
    return wut, wvtk, bbc, iden


def kernel(v, neighbors, W, b):
    from concourse.bass_utils import run_bass_kernel_spmd

    v = np.asarray(v, dtype=np.float32)
    neighbors = np.asarray(neighbors, dtype=np.float32)
    W = np.asarray(W, dtype=np.float32)
    b = np.asarray(b, dtype=np.float32)

    wut, wvtk, bbc, iden = _prep_weights(W, b)
    nc = _build()
    starts = _core_starts()
    in_maps = [
        {
            "nbr": np.ascontiguousarray(neighbors[:, s : s + NC_NODES, :]),
            "vin": np.ascontiguousarray(v[s : s + NC_NODES]),
            "wut": wut,
            "wvtk": wvtk,
            "bbc": bbc,
            "iden": iden,
        }
        for s in starts
    ]
    res = run_bass_kernel_spmd(nc, in_maps, core_ids=list(range(N_CORES)))

    out = np.empty((N_NODES, O), dtype=np.float32)
    step = N_NODES // N_CORES
    for c, s in enumerate(starts):
        own_lo = c * step
        own_hi = N_NODES if c == N_CORES - 1 else (c + 1) * step
        r = res.results[c]["out"]
        out[own_lo:own_hi] = r[own_lo - s : own_hi - s]
    return out



# revision 12
# speedup vs baseline: 6.5775x; 6.5775x over previous
"""Trainium2 Bass kernel for a GNN node-aggregator (fp16 pipeline).

Math (reference):
    out[n] = sum_k Linear(concat(v[n], u[k, n]))          with W = [Wv | Wu]
           = (sum_k u[k]) @ Wu.T  +  K * (v @ Wv.T)  +  K * b

The K-sum commutes with the linear layer, so the kernel streams the big
[K, N, D] neighbors tensor once.  Inputs are host-cast to fp16, halving
HBM traffic (the dominant cost).  The K-sum is split across engines so
no single engine is the bottleneck:

  - DVE sums KD of the 32 slabs with wide fp16 tensor_adds (2x mode),
  - PE transpose-accumulates the other KP slabs plus the DVE partial
    directly into PSUM as S^T via matmuls with an fp16 identity as the
    moving operand (regular matmuls -> start/stop accumulation works),
  - two small PE matmuls against host-preprocessed weights produce each
    128-node output block, DVE adds the bias, output DMAs out as fp16.

Each chunk's 32 k-slabs arrive in ONE ~7.2 MB DMA (128 partitions x
32 runs of 1792 contiguous bytes), so DMA fixed costs are negligible.
The chunk loop and the q-block loop are both software-pipelined with
lag 1 to keep every engine queue free of avoidable stalls.

Distribution: nodes sharded across 8 NeuronCores, 6272 = 49*128 nodes
per core (core slices overlap slightly; host gather keeps owned rows).
"""

import numpy as np

N_NODES = 50000
K_NB = 32
D = 128  # in features
O = 128  # out features
P = 128  # SBUF partitions

N_CORES = 8
QB = 49                # 128-node blocks per core
NC_NODES = P * QB      # 6272 nodes per core (overlapped shard)
CHUNK_Q = 7            # q-blocks per pipelined chunk
N_CHUNKS = QB // CHUNK_Q
KP = 18                # k-slabs summed on the tensor engine (PE)
# the other K_NB - KP slabs are summed on the vector engine (DVE)


def _core_starts():
    step = N_NODES // N_CORES
    return [min(c * step, N_NODES - NC_NODES) for c in range(N_CORES)]


def _build(repeats=1, kp=KP, chunk_q=CHUNK_Q, k_bufs=3, dual_ring=False,
           dma_only=False, loop_reps=1):
    """Build the per-core Bass program (SPMD: same NEFF on all cores)."""
    import concourse.mybir as mybir
    import concourse.tile as tile
    from concourse import bacc

    f32 = mybir.dt.float32
    f16 = mybir.dt.float16
    k_nb = K_NB
    qb = QB
    nc_nodes = P * qb
    n_chunks = qb // chunk_q
    assert qb % chunk_q == 0
    cw = chunk_q * D                   # chunk width in free elements
    dve_ks = list(range(kp, k_nb))    # slabs summed on DVE
    pe_ks = list(range(kp))           # slabs summed on PE
    assert len(dve_ks) >= 2

    nc = bacc.Bacc(trn_type="TRN2", name="node_aggregator")
    nbr = nc.dram_tensor("nbr", [k_nb, nc_nodes, D], f16, kind="ExternalInput")
    vin = nc.dram_tensor("vin", [nc_nodes, D], f16, kind="ExternalInput")
    wut = nc.dram_tensor("wut", [D, O], f16, kind="ExternalInput")    # Wu.T
    wvtk = nc.dram_tensor("wvtk", [D, O], f16, kind="ExternalInput")  # K * Wv.T
    bbc = nc.dram_tensor("bbc", [P, O], f32, kind="ExternalInput")    # K*b rows
    iden = nc.dram_tensor("iden", [P, P], f16, kind="ExternalInput")
    out = nc.dram_tensor("out", [nc_nodes, O], f16, kind="ExternalOutput")

    # Partition p holds nodes [qb*p, qb*p + qb): each chunk DMA is 128
    # partitions x 32 k-runs of chunk_q*D contiguous fp16 elements.
    nbr_r = nbr[:].rearrange("k (p q) d -> p k (q d)", p=P)
    v_r = vin[:].rearrange("(p q) d -> p (q d)", p=P)
    out_r = out[:].rearrange("(p q) o -> p (q o)", p=P)

    with tile.TileContext(nc) as tc, nc.allow_low_precision(
        reason="fp16 kernel; output tolerance is 2e-2"
    ):
        with (
            tc.tile_pool(name="cpool", bufs=1) as cpool,
            tc.tile_pool(name="kpool", bufs=k_bufs) as kpool,
            tc.tile_pool(name="spool", bufs=2) as spool,
            tc.tile_pool(name="tpool", bufs=4) as tpool,
            tc.tile_pool(name="opool", bufs=2) as opool,
            tc.tile_pool(name="pst", bufs=2, space="PSUM") as pst,
            tc.tile_pool(name="pvt", bufs=2, space="PSUM") as pvt,
            tc.tile_pool(name="pop", bufs=2, space="PSUM") as pop,
        ):
            # Constants + v + output ride the ACT HWDGE ring; the SP ring
            # is reserved for the big neighbor stream.
            wut_t = cpool.tile([D, O], f16)
            nc.scalar.dma_start(wut_t[:], wut[:])
            wvtk_t = cpool.tile([D, O], f16)
            nc.scalar.dma_start(wvtk_t[:], wvtk[:])
            bbc_t = cpool.tile([P, O], f32)
            nc.scalar.dma_start(bbc_t[:], bbc[:])
            iden_t = cpool.tile([P, P], f16)
            nc.scalar.dma_start(iden_t[:], iden[:])
            v_all = cpool.tile([P, qb * D], f16)
            nc.scalar.dma_start(v_all[:], v_r)

            def load_chunk(c):
                cs = slice(c * cw, (c + 1) * cw)
                big = kpool.tile([P, k_nb * cw], f16, tag="big")
                eng = nc.scalar if (dual_ring and c % 2) else nc.sync
                eng.dma_start(
                    big[:].rearrange("p (k f) -> p k f", k=k_nb), nbr_r[:, :, cs]
                )
                # DVE partial K-sum, wide fp16 adds (2x mode).
                S = spool.tile([P, cw], f16, tag="S")
                k0, k1 = dve_ks[0], dve_ks[1]
                nc.vector.tensor_add(
                    out=S[:],
                    in0=big[:, k0 * cw : (k0 + 1) * cw],
                    in1=big[:, k1 * cw : (k1 + 1) * cw],
                )
                for k in dve_ks[2:]:
                    nc.vector.tensor_add(
                        out=S[:], in0=S[:], in1=big[:, k * cw : (k + 1) * cw]
                    )
                return big, S

            def finals(st, vt, op_t, ot, qq):
                qs = slice(qq * D, (qq + 1) * D)
                nc.tensor.matmul(op_t[:], lhsT=st[:], rhs=wut_t[:], start=True, stop=False)
                nc.tensor.matmul(op_t[:], lhsT=vt[:], rhs=wvtk_t[:], start=False, stop=True)
                nc.vector.tensor_add(out=ot[:, qs], in0=op_t[:], in1=bbc_t[:])

            def pe_chunk(c, big, S):
                cs = slice(c * cw, (c + 1) * cw)
                ot = opool.tile([P, cw], f16, tag="ot")
                pending = None
                for qq in range(chunk_q):
                    gq = c * chunk_q + qq
                    # S^T accumulation for this q-block: PE slabs, then the
                    # DVE partial, all as matmuls with identity moving.
                    ST = pst.tile([D, P], f32, tag="ST")
                    for j, k in enumerate(pe_ks):
                        a = k * cw + qq * D
                        nc.tensor.matmul(
                            ST[:], lhsT=big[:, a : a + D], rhs=iden_t[:],
                            start=(j == 0), stop=False,
                        )
                    nc.tensor.matmul(
                        ST[:], lhsT=S[:, qq * D : (qq + 1) * D], rhs=iden_t[:],
                        start=False, stop=True,
                    )
                    VT = pvt.tile([D, P], f32, tag="VT")
                    nc.tensor.matmul(
                        VT[:], lhsT=v_all[:, gq * D : (gq + 1) * D], rhs=iden_t[:],
                        start=True, stop=True,
                    )
                    st = tpool.tile([D, P], f16, tag="st")
                    nc.scalar.copy(st[:], ST[:])
                    vt = tpool.tile([D, P], f16, tag="vt")
                    nc.scalar.copy(vt[:], VT[:])
                    if pending is not None:
                        finals(*pending)
                    op_t = pop.tile([P, O], f32, tag="OP")
                    pending = (st, vt, op_t, ot, qq)
                finals(*pending)
                nc.scalar.dma_start(out_r[:, cs], ot[:])

            def repeat_body():
                if dma_only:
                    # Pure-DMA roofline probe: stream neighbors, copy one
                    # slab slice back out so DCE keeps the transfers.
                    for c in range(n_chunks):
                        cs = slice(c * cw, (c + 1) * cw)
                        big = kpool.tile([P, k_nb * cw], f16, tag="big")
                        eng = nc.scalar if (dual_ring and c % 2) else nc.sync
                        eng.dma_start(
                            big[:].rearrange("p (k f) -> p k f", k=k_nb),
                            nbr_r[:, :, cs],
                        )
                        nc.scalar.dma_start(out_r[:, cs], big[:, 0:cw])
                    return
                prev = None
                for c in range(n_chunks):
                    cur = (c, *load_chunk(c))
                    if prev is not None:
                        pe_chunk(*prev)
                    prev = cur
                pe_chunk(*prev)

            if loop_reps > 1:
                # Hardware loop: constant instruction count at any repeat
                # count, for noise-proof (t_hi - t_lo) timing.
                with tc.For_i(0, loop_reps, 1):
                    for _ in range(repeats):
                        repeat_body()
            else:
                for _ in range(repeats):
                    repeat_body()
    nc.compile()
    return nc


def _prep_weights(W, b):
    W = np.asarray(W, dtype=np.float32)
    b = np.asarray(b, dtype=np.float32)
    Wv = W[:, :D]
    Wu = W[:, D:]
    wut = np.ascontiguousarray(Wu.T, dtype=np.float16)
    wvtk = np.ascontiguousarray((Wv.T * np.float32(K_NB)), dtype=np.float16)
    bbc = np.ascontiguousarray(
        np.broadcast_to((np.float32(K_NB) * b).astype(np.float32), (P, O))
    )
    iden = np.eye(P, dtype=np.float16)
    return wut, wvtk, bbc, iden


def _make_in_maps(v, neighbors, W, b):
    wut, wvtk, bbc, iden = _prep_weights(W, b)
    v16 = np.asarray(v).astype(np.float16)
    n16 = np.asarray(neighbors).astype(np.float16)
    return [
        {
            "nbr": np.ascontiguousarray(n16[:, s : s + NC_NODES, :]),
            "vin": np.ascontiguousarray(v16[s : s + NC_NODES]),
            "wut": wut,
            "wvtk": wvtk,
            "bbc": bbc,
            "iden": iden,
        }
        for s in _core_starts()
    ]


def kernel(v, neighbors, W, b):
    from concourse.bass_utils import run_bass_kernel_spmd

    in_maps = _make_in_maps(v, neighbors, W, b)
    nc = _build()
    res = run_bass_kernel_spmd(nc, in_maps, core_ids=list(range(N_CORES)))

    out = np.empty((N_NODES, O), dtype=np.float32)
    step = N_NODES // N_CORES
    for c, s in enumerate(_core_starts()):
        own_lo = c * step
        own_hi = N_NODES if c == N_CORES - 1 else (c + 1) * step
        r = np.asarray(res.results[c]["out"], dtype=np.float32)
        out[own_lo:own_hi] = r[own_lo - s : own_hi - s]
    return out


# revision 14
# speedup vs baseline: 6.6518x; 1.0113x over previous
"""Trainium2 Bass kernel for a GNN node-aggregator (fp16 pipeline).

Math (reference):
    out[n] = sum_k Linear(concat(v[n], u[k, n]))          with W = [Wv | Wu]
           = (sum_k u[k]) @ Wu.T  +  K * (v @ Wv.T)  +  K * b

The K-sum commutes with the linear layer, so the kernel streams the big
[K, N, D] neighbors tensor once.  Inputs are host-cast to fp16, halving
HBM traffic (the dominant cost).  The K-sum is split across engines so
no single engine is the bottleneck:

  - DVE sums KD of the 32 slabs with wide fp16 tensor_adds (2x mode),
  - PE transpose-accumulates the other KP slabs plus the DVE partial
    directly into PSUM as S^T via matmuls with an fp16 identity as the
    moving operand (regular matmuls -> start/stop accumulation works),
  - two small PE matmuls against host-preprocessed weights produce each
    128-node output block, DVE adds the bias, output DMAs out as fp16.

Each chunk's 32 k-slabs arrive in ONE ~7.2 MB DMA (128 partitions x
32 runs of 1792 contiguous bytes), so DMA fixed costs are negligible.
The chunk loop and the q-block loop are both software-pipelined with
lag 1 to keep every engine queue free of avoidable stalls.

Distribution: nodes sharded across 8 NeuronCores, 6272 = 49*128 nodes
per core (core slices overlap slightly; host gather keeps owned rows).
"""

import numpy as np

N_NODES = 50000
K_NB = 32
D = 128  # in features
O = 128  # out features
P = 128  # SBUF partitions

N_CORES = 8
QB = 49                # 128-node blocks per core
NC_NODES = P * QB      # 6272 nodes per core (overlapped shard)
CHUNK_Q = 7            # q-blocks per pipelined chunk
N_CHUNKS = QB // CHUNK_Q
KP = 18                # k-slabs summed on the tensor engine (PE)
# the other K_NB - KP slabs are summed on the vector engine (DVE)


def _core_starts():
    step = N_NODES // N_CORES
    return [min(c * step, N_NODES - NC_NODES) for c in range(N_CORES)]


def _build(repeats=1, kp=KP, chunk_q=CHUNK_Q, k_bufs=3, dual_ring=False,
           dma_only=False, loop_reps=1):
    """Build the per-core Bass program (SPMD: same NEFF on all cores)."""
    import concourse.mybir as mybir
    import concourse.tile as tile
    from concourse import bacc

    f32 = mybir.dt.float32
    f16 = mybir.dt.float16
    k_nb = K_NB
    qb = QB
    nc_nodes = P * qb
    n_chunks = qb // chunk_q
    assert qb % chunk_q == 0
    cw = chunk_q * D                   # chunk width in free elements
    dve_ks = list(range(kp, k_nb))    # slabs summed on DVE
    pe_ks = list(range(kp))           # slabs summed on PE
    assert len(dve_ks) >= 2

    nc = bacc.Bacc(trn_type="TRN2", name="node_aggregator")
    nbr = nc.dram_tensor("nbr", [k_nb, nc_nodes, D], f16, kind="ExternalInput")
    vin = nc.dram_tensor("vin", [nc_nodes, D], f16, kind="ExternalInput")
    wut = nc.dram_tensor("wut", [D, O], f16, kind="ExternalInput")    # Wu.T
    wvtk = nc.dram_tensor("wvtk", [D, O], f16, kind="ExternalInput")  # K * Wv.T
    bbc = nc.dram_tensor("bbc", [P, O], f32, kind="ExternalInput")    # K*b rows
    iden = nc.dram_tensor("iden", [P, P], f16, kind="ExternalInput")
    out = nc.dram_tensor("out", [nc_nodes, O], f16, kind="ExternalOutput")

    # Partition p holds nodes [qb*p, qb*p + qb): each chunk DMA is 128
    # partitions x 32 k-runs of chunk_q*D contiguous fp16 elements.
    nbr_r = nbr[:].rearrange("k (p q) d -> p k (q d)", p=P)
    v_r = vin[:].rearrange("(p q) d -> p (q d)", p=P)
    out_r = out[:].rearrange("(p q) o -> p (q o)", p=P)

    with tile.TileContext(nc) as tc, nc.allow_low_precision(
        reason="fp16 kernel; output tolerance is 2e-2"
    ):
        with (
            tc.tile_pool(name="cpool", bufs=1) as cpool,
            tc.tile_pool(name="kpool", bufs=k_bufs) as kpool,
            tc.tile_pool(name="spool", bufs=2) as spool,
            tc.tile_pool(name="tpool", bufs=4) as tpool,
            tc.tile_pool(name="opool", bufs=2) as opool,
            tc.tile_pool(name="pst", bufs=2, space="PSUM") as pst,
            tc.tile_pool(name="pvt", bufs=2, space="PSUM") as pvt,
            tc.tile_pool(name="pop", bufs=2, space="PSUM") as pop,
        ):
            # Constants + v + output ride the ACT HWDGE ring; the SP ring
            # is reserved for the big neighbor stream.
            wut_t = cpool.tile([D, O], f16)
            nc.scalar.dma_start(wut_t[:], wut[:])
            wvtk_t = cpool.tile([D, O], f16)
            nc.scalar.dma_start(wvtk_t[:], wvtk[:])
            bbc_t = cpool.tile([P, O], f32)
            nc.scalar.dma_start(bbc_t[:], bbc[:])
            iden_t = cpool.tile([P, P], f16)
            nc.scalar.dma_start(iden_t[:], iden[:])
            v_all = cpool.tile([P, qb * D], f16)
            nc.scalar.dma_start(v_all[:], v_r)

            kh = k_nb // 2  # half-split: PE work can start after half A lands

            def load_chunk(c):
                cs = slice(c * cw, (c + 1) * cw)
                big_a = kpool.tile([P, kh * cw], f16, tag="big_a")
                big_b = kpool.tile([P, kh * cw], f16, tag="big_b")
                eng = nc.scalar if (dual_ring and c % 2) else nc.sync
                eng.dma_start(
                    big_a[:].rearrange("p (k f) -> p k f", k=kh), nbr_r[:, :kh, cs]
                )
                eng.dma_start(
                    big_b[:].rearrange("p (k f) -> p k f", k=kh), nbr_r[:, kh:, cs]
                )

                def slab(k, lo, hi):
                    t = big_a if k < kh else big_b
                    return t[:, (k % kh) * cw + lo : (k % kh) * cw + hi]

                # DVE partial K-sum, wide fp16 adds (2x mode).
                S = spool.tile([P, cw], f16, tag="S")
                k0, k1 = dve_ks[0], dve_ks[1]
                nc.vector.tensor_add(
                    out=S[:], in0=slab(k0, 0, cw), in1=slab(k1, 0, cw)
                )
                for k in dve_ks[2:]:
                    nc.vector.tensor_add(out=S[:], in0=S[:], in1=slab(k, 0, cw))
                return slab, S

            def finals(st, vt, op_t, ot, qq):
                qs = slice(qq * D, (qq + 1) * D)
                nc.tensor.matmul(op_t[:], lhsT=st[:], rhs=wut_t[:], start=True, stop=False)
                nc.tensor.matmul(op_t[:], lhsT=vt[:], rhs=wvtk_t[:], start=False, stop=True)
                nc.vector.tensor_add(out=ot[:, qs], in0=op_t[:], in1=bbc_t[:])

            def pe_chunk(c, slab, S):
                cs = slice(c * cw, (c + 1) * cw)
                ot = opool.tile([P, cw], f16, tag="ot")
                pending = None
                for qq in range(chunk_q):
                    gq = c * chunk_q + qq
                    # S^T accumulation for this q-block: PE slabs, then the
                    # DVE partial, all as matmuls with identity moving.
                    ST = pst.tile([D, P], f32, tag="ST")
                    for j, k in enumerate(pe_ks):
                        nc.tensor.matmul(
                            ST[:], lhsT=slab(k, qq * D, (qq + 1) * D),
                            rhs=iden_t[:], start=(j == 0), stop=False,
                        )
                    nc.tensor.matmul(
                        ST[:], lhsT=S[:, qq * D : (qq + 1) * D], rhs=iden_t[:],
                        start=False, stop=True,
                    )
                    VT = pvt.tile([D, P], f32, tag="VT")
                    nc.tensor.matmul(
                        VT[:], lhsT=v_all[:, gq * D : (gq + 1) * D], rhs=iden_t[:],
                        start=True, stop=True,
                    )
                    st = tpool.tile([D, P], f16, tag="st")
                    nc.scalar.copy(st[:], ST[:])
                    vt = tpool.tile([D, P], f16, tag="vt")
                    nc.scalar.copy(vt[:], VT[:])
                    if pending is not None:
                        finals(*pending)
                    op_t = pop.tile([P, O], f32, tag="OP")
                    pending = (st, vt, op_t, ot, qq)
                finals(*pending)
                nc.scalar.dma_start(out_r[:, cs], ot[:])

            def repeat_body():
                if dma_only:
                    # Pure-DMA roofline probe: stream neighbors, copy one
                    # slab slice back out so DCE keeps the transfers.
                    for c in range(n_chunks):
                        cs = slice(c * cw, (c + 1) * cw)
                        big = kpool.tile([P, k_nb * cw], f16, tag="big")
                        eng = nc.scalar if (dual_ring and c % 2) else nc.sync
                        eng.dma_start(
                            big[:].rearrange("p (k f) -> p k f", k=k_nb),
                            nbr_r[:, :, cs],
                        )
                        nc.scalar.dma_start(out_r[:, cs], big[:, 0:cw])
                    return
                prev = None
                for c in range(n_chunks):
                    cur = (c, *load_chunk(c))
                    if prev is not None:
                        pe_chunk(*prev)
                    prev = cur
                pe_chunk(*prev)

            if loop_reps > 1:
                # Hardware loop: constant instruction count at any repeat
                # count, for noise-proof (t_hi - t_lo) timing.
                with tc.For_i(0, loop_reps, 1):
                    for _ in range(repeats):
                        repeat_body()
            else:
                for _ in range(repeats):
                    repeat_body()
    nc.compile()
    return nc


def _prep_weights(W, b):
    W = np.asarray(W, dtype=np.float32)
    b = np.asarray(b, dtype=np.float32)
    Wv = W[:, :D]
    Wu = W[:, D:]
    wut = np.ascontiguousarray(Wu.T, dtype=np.float16)
    wvtk = np.ascontiguousarray((Wv.T * np.float32(K_NB)), dtype=np.float16)
    bbc = np.ascontiguousarray(
        np.broadcast_to((np.float32(K_NB) * b).astype(np.float32), (P, O))
    )
    iden = np.eye(P, dtype=np.float16)
    return wut, wvtk, bbc, iden


def _make_in_maps(v, neighbors, W, b):
    wut, wvtk, bbc, iden = _prep_weights(W, b)
    v16 = np.asarray(v).astype(np.float16)
    n16 = np.asarray(neighbors).astype(np.float16)
    return [
        {
            "nbr": np.ascontiguousarray(n16[:, s : s + NC_NODES, :]),
            "vin": np.ascontiguousarray(v16[s : s + NC_NODES]),
            "wut": wut,
            "wvtk": wvtk,
            "bbc": bbc,
            "iden": iden,
        }
        for s in _core_starts()
    ]


def kernel(v, neighbors, W, b):
    from concourse.bass_utils import run_bass_kernel_spmd

    in_maps = _make_in_maps(v, neighbors, W, b)
    nc = _build()
    res = run_bass_kernel_spmd(nc, in_maps, core_ids=list(range(N_CORES)))

    out = np.empty((N_NODES, O), dtype=np.float32)
    step = N_NODES // N_CORES
    for c, s in enumerate(_core_starts()):
        own_lo = c * step
        own_hi = N_NODES if c == N_CORES - 1 else (c + 1) * step
        r = np.asarray(res.results[c]["out"], dtype=np.float32)
        out[own_lo:own_hi] = r[own_lo - s : own_hi - s]
    return out


# revision 32
# speedup vs baseline: 11.0194x; 1.6566x over previous
"""Trainium2 Bass kernel for a GNN node-aggregator (fp8 stream pipeline).

Math (reference):
    out[n] = sum_k Linear(concat(v[n], u[k, n]))          with W = [Wv | Wu]
           = (sum_k u[k]) @ Wu.T  +  K * (v @ Wv.T)  +  K * b

The K-sum commutes with the linear layer, so the kernel streams the big
[K, N, D] neighbors tensor exactly once.  Neighbors are host-cast to
fp8-e4m3 (4x less HBM traffic than f32): the output scale is dominated
by the K*(v @ Wv.T) term, so S-quantization noise dilutes to ~3.3e-3
relative error against the 2e-2 tolerance (verified by exact numpy
simulation AND on hardware).  v and the weights stay fp16.

The K-sum is split across engines so none becomes the bottleneck:
  - PE transpose-accumulates KP slabs plus the DVE partial directly
    into PSUM as S^T via matmuls with an fp8/fp16 identity as the
    moving operand (regular matmuls -> start/stop accumulation works),
  - DVE sums the other slabs as a pair tree (first level reads fp8 in
    1x mode, upper levels run fp16+fp16 in 2x mode),
  - per 128-node block: two fp16 matmuls apply Wu.T / K*Wv.T, the bias
    joins as a rank-1 matmul (ones x K*b) in the same PSUM group, and
    ACT cast-copies the result out as fp16.

Each chunk's slabs arrive in two group DMAs (PE slabs / DVE slabs; 128
partitions x 896 B contiguous runs) on the SP ring; v/consts/output
ride the ACT ring.  Chunk and q-block loops are software-pipelined with
lag 1.  Measured ~94 us/repeat vs a ~79 us pure-DMA floor (~340 GB/s
per core HBM limit).

Distribution: nodes sharded across 8 NeuronCores, 6272 = 49*128 nodes
per core (core slices overlap slightly; host gather keeps owned rows).
"""

import numpy as np

N_NODES = 50000
K_NB = 32
D = 128  # in features
O = 128  # out features
P = 128  # SBUF partitions

N_CORES = 8
QB = 49                # 128-node blocks per core
NC_NODES = P * QB      # 6272 nodes per core (overlapped shard)
CHUNK_Q = 7            # q-blocks per pipelined chunk
N_CHUNKS = QB // CHUNK_Q
KP = 20                # k-slabs summed on the tensor engine (PE)
# the other K_NB - KP slabs are summed on the vector engine (DVE)


def _core_starts():
    step = N_NODES // N_CORES
    return [min(c * step, N_NODES - NC_NODES) for c in range(N_CORES)]


def _build(repeats=1, kp=KP, chunk_q=CHUNK_Q, k_bufs=3, dual_ring=False,
           dma_only=False, loop_reps=1):
    """Build the per-core Bass program (SPMD: same NEFF on all cores)."""
    import concourse.mybir as mybir
    import concourse.tile as tile
    from concourse import bacc

    f32 = mybir.dt.float32
    f16 = mybir.dt.float16
    f8 = mybir.dt.float8e4
    k_nb = K_NB
    qb = QB
    nc_nodes = P * qb
    n_chunks = qb // chunk_q
    assert qb % chunk_q == 0
    cw = chunk_q * D                   # chunk width in free elements
    dve_ks = list(range(kp, k_nb))    # slabs summed on DVE
    pe_ks = list(range(kp))           # slabs summed on PE
    assert len(dve_ks) >= 2

    nc = bacc.Bacc(trn_type="TRN2", name="node_aggregator")
    nbr = nc.dram_tensor("nbr", [k_nb, nc_nodes, D], f8, kind="ExternalInput")
    vin = nc.dram_tensor("vin", [nc_nodes, D], f16, kind="ExternalInput")
    wut = nc.dram_tensor("wut", [D, O], f16, kind="ExternalInput")    # Wu.T
    wvtk = nc.dram_tensor("wvtk", [D, O], f16, kind="ExternalInput")  # K * Wv.T
    bbc = nc.dram_tensor("bbc", [1, O], f16, kind="ExternalInput")    # K*b row
    ones = nc.dram_tensor("ones", [1, P], f16, kind="ExternalInput")
    iden = nc.dram_tensor("iden", [P, P], f16, kind="ExternalInput")
    iden8 = nc.dram_tensor("iden8", [P, P], f8, kind="ExternalInput")
    out = nc.dram_tensor("out", [nc_nodes, O], f16, kind="ExternalOutput")

    # Partition p holds nodes [qb*p, qb*p + qb): each chunk DMA is 128
    # partitions x 32 k-runs of chunk_q*D contiguous fp16 elements.
    nbr_r = nbr[:].rearrange("k (p q) d -> p k (q d)", p=P)
    v_r = vin[:].rearrange("(p q) d -> p (q d)", p=P)
    out_r = out[:].rearrange("(p q) o -> p (q o)", p=P)

    with tile.TileContext(nc) as tc, nc.allow_low_precision(
        reason="fp16 kernel; output tolerance is 2e-2"
    ):
        with (
            tc.tile_pool(name="cpool", bufs=1) as cpool,
            tc.tile_pool(name="kpool", bufs=k_bufs) as kpool,
            tc.tile_pool(name="spool", bufs=12) as spool,
            tc.tile_pool(name="tpool", bufs=4) as tpool,
            tc.tile_pool(name="opool", bufs=2) as opool,
            tc.tile_pool(name="pst", bufs=2, space="PSUM") as pst,
            tc.tile_pool(name="pvt", bufs=2, space="PSUM") as pvt,
            tc.tile_pool(name="pop", bufs=2, space="PSUM") as pop,
        ):
            # Constants + v + output ride the ACT HWDGE ring; the SP ring
            # is reserved for the big neighbor stream.
            wut_t = cpool.tile([D, O], f16)
            nc.scalar.dma_start(wut_t[:], wut[:])
            wvtk_t = cpool.tile([D, O], f16)
            nc.scalar.dma_start(wvtk_t[:], wvtk[:])
            bbc_t = cpool.tile([1, O], f16)
            nc.scalar.dma_start(bbc_t[:], bbc[:])
            ones_t = cpool.tile([1, P], f16)
            nc.scalar.dma_start(ones_t[:], ones[:])
            iden_t = cpool.tile([P, P], f16)
            nc.scalar.dma_start(iden_t[:], iden[:])
            iden8_t = cpool.tile([P, P], f8)
            nc.scalar.dma_start(iden8_t[:], iden8[:])
            v_all = cpool.tile([P, qb * D], f16)
            nc.scalar.dma_start(v_all[:], v_r)

            kd = k_nb - kp  # group-split: PE work can start once big_a lands

            def load_chunk(c):
                cs = slice(c * cw, (c + 1) * cw)
                big_a = kpool.tile([P, kp * cw], f8, tag="big_a")
                big_b = kpool.tile([P, kd * cw], f8, tag="big_b")
                eng = nc.scalar if (dual_ring and c % 2) else nc.sync
                eng.dma_start(
                    big_a[:].rearrange("p (k f) -> p k f", k=kp), nbr_r[:, :kp, cs]
                )
                eng.dma_start(
                    big_b[:].rearrange("p (k f) -> p k f", k=kd), nbr_r[:, kp:, cs]
                )

                def slab(k, lo, hi):
                    t = big_a if k < kp else big_b
                    return t[:, (k % kp) * cw + lo : (k % kp) * cw + hi]

                # DVE partial K-sum of the fp8 slabs, as a pair tree: the
                # first level reads 1-byte operands (1x mode); the upper
                # levels are fp16+fp16 in 2x mode.
                level = []
                ks = list(dve_ks)
                if len(ks) % 2:
                    t = spool.tile([P, cw], f16, tag="tp")
                    nc.vector.tensor_copy(out=t[:], in_=slab(ks[0], 0, cw))
                    level.append(t)
                    ks = ks[1:]
                for a, b2 in zip(ks[::2], ks[1::2]):
                    t = spool.tile([P, cw], f16, tag="tp")
                    nc.vector.tensor_add(
                        out=t[:], in0=slab(a, 0, cw), in1=slab(b2, 0, cw)
                    )
                    level.append(t)
                while len(level) > 1:
                    nxt = []
                    for i in range(0, len(level) - 1, 2):
                        t = spool.tile([P, cw], f16, tag="tp")
                        nc.vector.tensor_add(
                            out=t[:], in0=level[i][:], in1=level[i + 1][:]
                        )
                        nxt.append(t)
                    if len(level) % 2:
                        nxt.append(level[-1])
                    level = nxt
                return slab, level[0]

            def finals(st, vt, op_t, ot, qq):
                qs = slice(qq * D, (qq + 1) * D)
                nc.tensor.matmul(op_t[:], lhsT=st[:], rhs=wut_t[:], start=True, stop=False)
                nc.tensor.matmul(op_t[:], lhsT=vt[:], rhs=wvtk_t[:], start=False, stop=False)
                # bias as a rank-1 matmul: OP[n, o] += ones[n] * (K*b)[o]
                nc.tensor.matmul(op_t[:], lhsT=ones_t[:], rhs=bbc_t[:], start=False, stop=True)
                nc.scalar.copy(ot[:, qs], op_t[:])

            def pe_chunk(c, slab, S):
                cs = slice(c * cw, (c + 1) * cw)
                ot = opool.tile([P, cw], f16, tag="ot")
                pending = None
                for qq in range(chunk_q):
                    gq = c * chunk_q + qq
                    # S^T accumulation for this q-block: PE slabs, then the
                    # DVE partial, all as matmuls with identity moving.
                    ST = pst.tile([D, P], f32, tag="ST")
                    for j, k in enumerate(pe_ks):
                        nc.tensor.matmul(
                            ST[:], lhsT=slab(k, qq * D, (qq + 1) * D),
                            rhs=iden8_t[:], start=(j == 0), stop=False,
                        )
                    nc.tensor.matmul(
                        ST[:], lhsT=S[:, qq * D : (qq + 1) * D], rhs=iden_t[:],
                        start=False, stop=True,
                    )
                    VT = pvt.tile([D, P], f32, tag="VT")
                    nc.tensor.matmul(
                        VT[:], lhsT=v_all[:, gq * D : (gq + 1) * D], rhs=iden_t[:],
                        start=True, stop=True,
                    )
                    st = tpool.tile([D, P], f16, tag="st")
                    nc.scalar.copy(st[:], ST[:])
                    vt = tpool.tile([D, P], f16, tag="vt")
                    nc.scalar.copy(vt[:], VT[:])
                    if pending is not None:
                        finals(*pending)
                    op_t = pop.tile([P, O], f32, tag="OP")
                    pending = (st, vt, op_t, ot, qq)
                finals(*pending)
                nc.scalar.dma_start(out_r[:, cs], ot[:])

            def repeat_body():
                if dma_only:
                    # Pure-DMA roofline probe: stream neighbors, copy one
                    # slab slice back out so DCE keeps the transfers.
                    for c in range(n_chunks):
                        cs = slice(c * cw, (c + 1) * cw)
                        big = kpool.tile([P, k_nb * cw], f8, tag="big")
                        eng = nc.scalar if (dual_ring and c % 2) else nc.sync
                        eng.dma_start(
                            big[:].rearrange("p (k f) -> p k f", k=k_nb),
                            nbr_r[:, :, cs],
                        )
                        nc.scalar.dma_start(
                            out_r[:, cs], big[:, 0 : 2 * cw].bitcast(f16)
                        )
                    return
                prev = None
                for c in range(n_chunks):
                    cur = (c, *load_chunk(c))
                    if prev is not None:
                        pe_chunk(*prev)
                    prev = cur
                pe_chunk(*prev)

            if loop_reps > 1:
                # Hardware loop: constant instruction count at any repeat
                # count, for noise-proof (t_hi - t_lo) timing.
                with tc.For_i(0, loop_reps, 1):
                    for _ in range(repeats):
                        repeat_body()
            else:
                for _ in range(repeats):
                    repeat_body()
    nc.compile()
    return nc


def _f8np():
    import concourse.mybir as mybir

    return mybir.dt.np(mybir.dt.float8e4)


def _prep_weights(W, b):
    W = np.asarray(W, dtype=np.float32)
    b = np.asarray(b, dtype=np.float32)
    Wv = W[:, :D]
    Wu = W[:, D:]
    wut = np.ascontiguousarray(Wu.T, dtype=np.float16)
    wvtk = np.ascontiguousarray((Wv.T * np.float32(K_NB)), dtype=np.float16)
    bbc = np.ascontiguousarray((np.float32(K_NB) * b).astype(np.float16))[None, :]
    ones = np.ones((1, P), dtype=np.float16)
    iden = np.eye(P, dtype=np.float16)
    iden8 = np.eye(P, dtype=_f8np())
    return wut, wvtk, bbc, ones, iden, iden8


def _make_in_maps(v, neighbors, W, b):
    wut, wvtk, bbc, ones, iden, iden8 = _prep_weights(W, b)
    v16 = np.asarray(v).astype(np.float16)
    n8 = np.asarray(neighbors).astype(_f8np())
    return [
        {
            "nbr": np.ascontiguousarray(n8[:, s : s + NC_NODES, :]),
            "vin": np.ascontiguousarray(v16[s : s + NC_NODES]),
            "wut": wut,
            "wvtk": wvtk,
            "bbc": bbc,
            "ones": ones,
            "iden": iden,
            "iden8": iden8,
        }
        for s in _core_starts()
    ]


def kernel(v, neighbors, W, b):
    from concourse.bass_utils import run_bass_kernel_spmd

    in_maps = _make_in_maps(v, neighbors, W, b)
    nc = _build()
    res = run_bass_kernel_spmd(nc, in_maps, core_ids=list(range(N_CORES)))

    out = np.empty((N_NODES, O), dtype=np.float32)
    step = N_NODES // N_CORES
    for c, s in enumerate(_core_starts()):
        own_lo = c * step
        own_hi = N_NODES if c == N_CORES - 1 else (c + 1) * step
        r = np.asarray(res.results[c]["out"], dtype=np.float32)
        out[own_lo:own_hi] = r[own_lo - s : own_hi - s]
    return out


# revision 35
# speedup vs baseline: 11.9838x; 1.0875x over previous
"""Trainium2 Bass kernel for a GNN node-aggregator (fp8 stream pipeline).

Math (reference):
    out[n] = sum_k Linear(concat(v[n], u[k, n]))          with W = [Wv | Wu]
           = (sum_k u[k]) @ Wu.T  +  K * (v @ Wv.T)  +  K * b

The K-sum commutes with the linear layer, so the kernel streams the big
[K, N, D] neighbors tensor exactly once.  Neighbors are host-cast to
fp8-e4m3 (4x less HBM traffic than f32): the output scale is dominated
by the K*(v @ Wv.T) term, so S-quantization noise dilutes to ~3.3e-3
relative error against the 2e-2 tolerance (verified by exact numpy
simulation AND on hardware).  v and the weights stay fp16.

The K-sum is split across engines so none becomes the bottleneck:
  - PE transpose-accumulates KP slabs plus the DVE partial directly
    into PSUM as S^T via matmuls with an fp8/fp16 identity as the
    moving operand (regular matmuls -> start/stop accumulation works),
  - DVE sums the other slabs as a pair tree (first level reads fp8 in
    1x mode, upper levels run fp16+fp16 in 2x mode),
  - per 128-node block: two fp16 matmuls apply Wu.T / K*Wv.T, the bias
    joins as a rank-1 matmul (ones x K*b) in the same PSUM group, and
    ACT cast-copies the result out as fp16.

Each chunk's slabs arrive in two group DMAs (PE slabs / DVE slabs; 128
partitions x 896 B contiguous runs) on the SP ring; v/consts/output
ride the ACT ring.  Chunk and q-block loops are software-pipelined with
lag 1.  Measured ~94 us/repeat vs a ~79 us pure-DMA floor (~340 GB/s
per core HBM limit).

Distribution: nodes sharded across 8 NeuronCores, 6272 = 49*128 nodes
per core (core slices overlap slightly; host gather keeps owned rows).
"""

import numpy as np

N_NODES = 50000
K_NB = 32
D = 128  # in features
O = 128  # out features
P = 128  # SBUF partitions

N_CORES = 8
QB = 49                # 128-node blocks per core
NC_NODES = P * QB      # 6272 nodes per core (overlapped shard)
CHUNK_Q = 7            # q-blocks per pipelined chunk
N_CHUNKS = QB // CHUNK_Q
KP = 20                # k-slabs summed on the tensor engine (PE)
# the other K_NB - KP slabs are summed on the vector engine (DVE)


def _core_starts():
    step = N_NODES // N_CORES
    return [min(c * step, N_NODES - NC_NODES) for c in range(N_CORES)]


def _build(repeats=1, kp=KP, chunk_q=CHUNK_Q, k_bufs=3, dual_ring=False,
           dma_only=False, loop_reps=1):
    """Build the per-core Bass program (SPMD: same NEFF on all cores)."""
    import concourse.mybir as mybir
    import concourse.tile as tile
    from concourse import bacc

    f32 = mybir.dt.float32
    f16 = mybir.dt.float16
    f8 = mybir.dt.float8e4
    k_nb = K_NB
    qb = QB
    nc_nodes = P * qb
    n_chunks = qb // chunk_q
    assert qb % chunk_q == 0
    cw = chunk_q * D                   # chunk width in free elements
    dve_ks = list(range(kp, k_nb))    # slabs summed on DVE
    pe_ks = list(range(kp))           # slabs summed on PE
    assert len(dve_ks) >= 2

    nc = bacc.Bacc(trn_type="TRN2", name="node_aggregator")
    nbr = nc.dram_tensor("nbr", [k_nb, nc_nodes, D], f8, kind="ExternalInput")
    vin = nc.dram_tensor("vin", [nc_nodes, D], f16, kind="ExternalInput")
    wut = nc.dram_tensor("wut", [D, O], f16, kind="ExternalInput")    # Wu.T
    wvtk = nc.dram_tensor("wvtk", [D, O], f16, kind="ExternalInput")  # K * Wv.T
    bbc = nc.dram_tensor("bbc", [1, O], f16, kind="ExternalInput")    # K*b row
    ones = nc.dram_tensor("ones", [1, P], f16, kind="ExternalInput")
    iden = nc.dram_tensor("iden", [P, P], f16, kind="ExternalInput")
    iden8 = nc.dram_tensor("iden8", [P, P], f8, kind="ExternalInput")
    out = nc.dram_tensor("out", [nc_nodes, O], f16, kind="ExternalOutput")

    # Partition p holds nodes [qb*p, qb*p + qb): each chunk DMA is 128
    # partitions x 32 k-runs of chunk_q*D contiguous fp16 elements.
    nbr_r = nbr[:].rearrange("k (p q) d -> p k (q d)", p=P)
    out_r = out[:].rearrange("(p q) o -> p (q o)", p=P)

    with tile.TileContext(nc) as tc, nc.allow_low_precision(
        reason="fp16 kernel; output tolerance is 2e-2"
    ):
        with (
            tc.tile_pool(name="cpool", bufs=1) as cpool,
            tc.tile_pool(name="kpool", bufs=k_bufs) as kpool,
            tc.tile_pool(name="spool", bufs=12) as spool,
            tc.tile_pool(name="tpool", bufs=4) as tpool,
            tc.tile_pool(name="opool", bufs=2) as opool,
            tc.tile_pool(name="pst", bufs=2, space="PSUM") as pst,
            tc.tile_pool(name="pop", bufs=2, space="PSUM") as pop,
        ):
            # Constants + v + output ride the ACT HWDGE ring; the SP ring
            # is reserved for the big neighbor stream.
            wut_t = cpool.tile([D, O], f16)
            nc.scalar.dma_start(wut_t[:], wut[:])
            wvtk_t = cpool.tile([D, O], f16)
            nc.scalar.dma_start(wvtk_t[:], wvtk[:])
            bbc_t = cpool.tile([1, O], f16)
            nc.scalar.dma_start(bbc_t[:], bbc[:])
            ones_t = cpool.tile([1, P], f16)
            nc.scalar.dma_start(ones_t[:], ones[:])
            iden_t = cpool.tile([P, P], f16)
            nc.scalar.dma_start(iden_t[:], iden[:])
            iden8_t = cpool.tile([P, P], f8)
            nc.scalar.dma_start(iden8_t[:], iden8[:])
            # v arrives host-permuted to q-block-major node order, so one
            # xbar DMA-transpose gives v^T with every q-block's 128 node
            # columns contiguous; kills the per-q-block PE transpose and
            # its PSUM->SBUF copy entirely.
            vt_all = cpool.tile([P, nc_nodes], f16)
            nc.scalar.dma_start_transpose(vt_all[:], vin[:])

            kd = k_nb - kp  # group-split: PE work can start once big_a lands

            def load_chunk(c):
                cs = slice(c * cw, (c + 1) * cw)
                big_a = kpool.tile([P, kp * cw], f8, tag="big_a")
                big_b = kpool.tile([P, kd * cw], f8, tag="big_b")
                eng = nc.scalar if (dual_ring and c % 2) else nc.sync
                eng.dma_start(
                    big_a[:].rearrange("p (k f) -> p k f", k=kp), nbr_r[:, :kp, cs]
                )
                eng.dma_start(
                    big_b[:].rearrange("p (k f) -> p k f", k=kd), nbr_r[:, kp:, cs]
                )

                def slab(k, lo, hi):
                    t = big_a if k < kp else big_b
                    return t[:, (k % kp) * cw + lo : (k % kp) * cw + hi]

                # DVE partial K-sum of the fp8 slabs, as a pair tree: the
                # first level reads 1-byte operands (1x mode); the upper
                # levels are fp16+fp16 in 2x mode.
                level = []
                ks = list(dve_ks)
                if len(ks) % 2:
                    t = spool.tile([P, cw], f16, tag="tp")
                    nc.vector.tensor_copy(out=t[:], in_=slab(ks[0], 0, cw))
                    level.append(t)
                    ks = ks[1:]
                for a, b2 in zip(ks[::2], ks[1::2]):
                    t = spool.tile([P, cw], f16, tag="tp")
                    nc.vector.tensor_add(
                        out=t[:], in0=slab(a, 0, cw), in1=slab(b2, 0, cw)
                    )
                    level.append(t)
                while len(level) > 1:
                    nxt = []
                    for i in range(0, len(level) - 1, 2):
                        t = spool.tile([P, cw], f16, tag="tp")
                        nc.vector.tensor_add(
                            out=t[:], in0=level[i][:], in1=level[i + 1][:]
                        )
                        nxt.append(t)
                    if len(level) % 2:
                        nxt.append(level[-1])
                    level = nxt
                return slab, level[0]

            # q-blocks are processed in groups of up to 4 sharing one PSUM
            # bank, so ACT does a few wide PSUM->SBUF copies per chunk
            # instead of one small copy per q-block.
            GRP = 4
            groups = [
                (g0, min(g0 + GRP, chunk_q)) for g0 in range(0, chunk_q, GRP)
            ]

            def finals(c, g0, g1, stb, ot):
                opb = pop.tile([P, GRP * O], f32, tag="OP")
                for qq in range(g0, g1):
                    off = (qq - g0) * O
                    gq = c * chunk_q + qq
                    os_ = slice(off, off + O)
                    nc.tensor.matmul(
                        opb[:, os_], lhsT=stb[:, off : off + P], rhs=wut_t[:],
                        start=True, stop=False,
                    )
                    nc.tensor.matmul(
                        opb[:, os_], lhsT=vt_all[:, gq * D : (gq + 1) * D],
                        rhs=wvtk_t[:], start=False, stop=False,
                    )
                    # bias as a rank-1 matmul: OP[n, o] += ones[n] * (K*b)[o]
                    nc.tensor.matmul(
                        opb[:, os_], lhsT=ones_t[:], rhs=bbc_t[:],
                        start=False, stop=True,
                    )
                nc.scalar.copy(ot[:, g0 * O : g1 * O], opb[:, : (g1 - g0) * O])

            def pe_chunk(c, slab, S):
                cs = slice(c * cw, (c + 1) * cw)
                ot = opool.tile([P, cw], f16, tag="ot")
                pending = None
                for g0, g1 in groups:
                    gw = (g1 - g0) * P
                    # S^T accumulation, one PSUM-bank column range per
                    # q-block: PE slabs, then the DVE partial, all as
                    # matmuls with identity moving.
                    STb = pst.tile([D, GRP * P], f32, tag="ST")
                    for qq in range(g0, g1):
                        off = (qq - g0) * P
                        ss = slice(off, off + P)
                        for j, k in enumerate(pe_ks):
                            nc.tensor.matmul(
                                STb[:, ss], lhsT=slab(k, qq * D, (qq + 1) * D),
                                rhs=iden8_t[:], start=(j == 0), stop=False,
                            )
                        nc.tensor.matmul(
                            STb[:, ss], lhsT=S[:, qq * D : (qq + 1) * D],
                            rhs=iden_t[:], start=False, stop=True,
                        )
                    stb = tpool.tile([D, GRP * P], f16, tag="st")
                    nc.scalar.copy(stb[:, :gw], STb[:, :gw])
                    if pending is not None:
                        finals(*pending)
                    pending = (c, g0, g1, stb, ot)
                finals(*pending)
                nc.scalar.dma_start(out_r[:, cs], ot[:])

            def repeat_body():
                if dma_only:
                    # Pure-DMA roofline probe: stream neighbors, copy one
                    # slab slice back out so DCE keeps the transfers.
                    for c in range(n_chunks):
                        cs = slice(c * cw, (c + 1) * cw)
                        big = kpool.tile([P, k_nb * cw], f8, tag="big")
                        eng = nc.scalar if (dual_ring and c % 2) else nc.sync
                        eng.dma_start(
                            big[:].rearrange("p (k f) -> p k f", k=k_nb),
                            nbr_r[:, :, cs],
                        )
                        nc.scalar.dma_start(
                            out_r[:, cs], big[:, 0 : 2 * cw].bitcast(f16)
                        )
                    return
                prev = None
                for c in range(n_chunks):
                    cur = (c, *load_chunk(c))
                    if prev is not None:
                        pe_chunk(*prev)
                    prev = cur
                pe_chunk(*prev)

            if loop_reps > 1:
                # Hardware loop: constant instruction count at any repeat
                # count, for noise-proof (t_hi - t_lo) timing.
                with tc.For_i(0, loop_reps, 1):
                    for _ in range(repeats):
                        repeat_body()
            else:
                for _ in range(repeats):
                    repeat_body()
    nc.compile()
    return nc


def _f8np():
    import concourse.mybir as mybir

    return mybir.dt.np(mybir.dt.float8e4)


def _prep_weights(W, b):
    W = np.asarray(W, dtype=np.float32)
    b = np.asarray(b, dtype=np.float32)
    Wv = W[:, :D]
    Wu = W[:, D:]
    wut = np.ascontiguousarray(Wu.T, dtype=np.float16)
    wvtk = np.ascontiguousarray((Wv.T * np.float32(K_NB)), dtype=np.float16)
    bbc = np.ascontiguousarray((np.float32(K_NB) * b).astype(np.float16))[None, :]
    ones = np.ones((1, P), dtype=np.float16)
    iden = np.eye(P, dtype=np.float16)
    iden8 = np.eye(P, dtype=_f8np())
    return wut, wvtk, bbc, ones, iden, iden8


def _make_in_maps(v, neighbors, W, b):
    wut, wvtk, bbc, ones, iden, iden8 = _prep_weights(W, b)
    v16 = np.asarray(v).astype(np.float16)
    n8 = np.asarray(neighbors).astype(_f8np())
    return [
        {
            "nbr": np.ascontiguousarray(n8[:, s : s + NC_NODES, :]),
            # q-block-major node order (node p*QB+q -> row q*P+p), so the
            # on-device xbar transpose yields contiguous q-block columns
            "vin": np.ascontiguousarray(
                v16[s : s + NC_NODES]
                .reshape(P, QB, D)
                .transpose(1, 0, 2)
                .reshape(NC_NODES, D)
            ),
            "wut": wut,
            "wvtk": wvtk,
            "bbc": bbc,
            "ones": ones,
            "iden": iden,
            "iden8": iden8,
        }
        for s in _core_starts()
    ]


def kernel(v, neighbors, W, b):
    from concourse.bass_utils import run_bass_kernel_spmd

    in_maps = _make_in_maps(v, neighbors, W, b)
    nc = _build()
    res = run_bass_kernel_spmd(nc, in_maps, core_ids=list(range(N_CORES)))

    out = np.empty((N_NODES, O), dtype=np.float32)
    step = N_NODES // N_CORES
    for c, s in enumerate(_core_starts()):
        own_lo = c * step
        own_hi = N_NODES if c == N_CORES - 1 else (c + 1) * step
        r = np.asarray(res.results[c]["out"], dtype=np.float32)
        out[own_lo:own_hi] = r[own_lo - s : own_hi - s]
    return out


# revision 38
# speedup vs baseline: 13.6121x; 1.1359x over previous
"""Trainium2 Bass kernel for a GNN node-aggregator (fp8 stream pipeline).

Math (reference):
    out[n] = sum_k Linear(concat(v[n], u[k, n]))          with W = [Wv | Wu]
           = (sum_k u[k]) @ Wu.T  +  K * (v @ Wv.T)  +  K * b

The K-sum commutes with the linear layer, so the kernel streams the big
[K, N, D] neighbors tensor exactly once.  Neighbors are host-cast to
fp8-e4m3 (4x less HBM traffic than f32): the output scale is dominated
by the K*(v @ Wv.T) term, so S-quantization noise dilutes to ~3.3e-3
relative error against the 2e-2 tolerance (verified by exact numpy
simulation AND on hardware).  v and the weights stay fp16.

The K-sum is split across engines so none becomes the bottleneck:
  - PE transpose-accumulates KP slabs plus the DVE partial directly
    into PSUM as S^T via matmuls with an fp8/fp16 identity as the
    moving operand (regular matmuls -> start/stop accumulation works),
  - DVE sums the other slabs as a pair tree (first level reads fp8 in
    1x mode, upper levels run fp16+fp16 in 2x mode),
  - per 128-node block: two fp16 matmuls apply Wu.T / K*Wv.T, the bias
    joins as a rank-1 matmul (ones x K*b) in the same PSUM group, and
    ACT cast-copies the result out as fp16.

Each chunk's slabs arrive in two group DMAs (PE slabs / DVE slabs; 128
partitions x 896 B contiguous runs) on the SP ring; v/consts/output
ride the ACT ring.  Chunk and q-block loops are software-pipelined with
lag 1.  Measured ~94 us/repeat vs a ~79 us pure-DMA floor (~340 GB/s
per core HBM limit).

Distribution: nodes sharded across 8 NeuronCores, 6272 = 49*128 nodes
per core (core slices overlap slightly; host gather keeps owned rows).
"""

import numpy as np

N_NODES = 50000
K_NB = 32
D = 128  # in features
O = 128  # out features
P = 128  # SBUF partitions

N_CORES = 8
QB = 49                # 128-node blocks per core
NC_NODES = P * QB      # 6272 nodes per core (overlapped shard)
CHUNK_Q = 7            # q-blocks per pipelined chunk
N_CHUNKS = QB // CHUNK_Q
KP = 20                # k-slabs summed on the tensor engine (PE)
# the other K_NB - KP slabs are summed on the vector engine (DVE)


def _core_starts():
    step = N_NODES // N_CORES
    return [min(c * step, N_NODES - NC_NODES) for c in range(N_CORES)]


def _build(repeats=1, kp=KP, chunk_q=CHUNK_Q, k_bufs=4, dual_ring=False,
           dma_only=False, loop_reps=1):
    """Build the per-core Bass program (SPMD: same NEFF on all cores)."""
    import concourse.mybir as mybir
    import concourse.tile as tile
    from concourse import bacc

    f32 = mybir.dt.float32
    f16 = mybir.dt.float16
    f8 = mybir.dt.float8e4
    k_nb = K_NB
    qb = QB
    nc_nodes = P * qb
    n_chunks = qb // chunk_q
    assert qb % chunk_q == 0
    cw = chunk_q * D                   # chunk width in free elements
    dve_ks = list(range(kp, k_nb))    # slabs summed on DVE
    pe_ks = list(range(kp))           # slabs summed on PE
    assert len(dve_ks) >= 2

    nc = bacc.Bacc(trn_type="TRN2", name="node_aggregator")
    nbr = nc.dram_tensor("nbr", [k_nb, nc_nodes, D], f8, kind="ExternalInput")
    vin = nc.dram_tensor("vin", [nc_nodes, D], f16, kind="ExternalInput")
    wut = nc.dram_tensor("wut", [D, O], f16, kind="ExternalInput")    # Wu.T
    wvtk = nc.dram_tensor("wvtk", [D, O], f16, kind="ExternalInput")  # K * Wv.T
    bbc = nc.dram_tensor("bbc", [1, O], f16, kind="ExternalInput")    # K*b row
    ones = nc.dram_tensor("ones", [1, P], f16, kind="ExternalInput")
    iden = nc.dram_tensor("iden", [P, P], f16, kind="ExternalInput")
    iden8 = nc.dram_tensor("iden8", [P, P], f8, kind="ExternalInput")
    out = nc.dram_tensor("out", [nc_nodes, O], f16, kind="ExternalOutput")

    # Partition p holds nodes [qb*p, qb*p + qb): each chunk DMA is 128
    # partitions x 32 k-runs of chunk_q*D contiguous fp16 elements.
    nbr_r = nbr[:].rearrange("k (p q) d -> p k (q d)", p=P)
    out_r = out[:].rearrange("(p q) o -> p (q o)", p=P)

    with tile.TileContext(nc) as tc, nc.allow_low_precision(
        reason="fp16 kernel; output tolerance is 2e-2"
    ):
        with (
            tc.tile_pool(name="cpool", bufs=1) as cpool,
            tc.tile_pool(name="kpool", bufs=k_bufs) as kpool,
            tc.tile_pool(name="spool", bufs=12) as spool,
            tc.tile_pool(name="tpool", bufs=4) as tpool,
            tc.tile_pool(name="opool", bufs=2) as opool,
            tc.tile_pool(name="pst", bufs=2, space="PSUM") as pst,
            tc.tile_pool(name="pop", bufs=2, space="PSUM") as pop,
        ):
            # Constants + v + output ride the ACT HWDGE ring; the SP ring
            # is reserved for the big neighbor stream.
            wut_t = cpool.tile([D, O], f16)
            nc.scalar.dma_start(wut_t[:], wut[:])
            wvtk_t = cpool.tile([D, O], f16)
            nc.scalar.dma_start(wvtk_t[:], wvtk[:])
            bbc_t = cpool.tile([1, O], f16)
            nc.scalar.dma_start(bbc_t[:], bbc[:])
            ones_t = cpool.tile([1, P], f16)
            nc.scalar.dma_start(ones_t[:], ones[:])
            iden_t = cpool.tile([P, P], f16)
            nc.scalar.dma_start(iden_t[:], iden[:])
            iden8_t = cpool.tile([P, P], f8)
            nc.scalar.dma_start(iden8_t[:], iden8[:])
            # v arrives host-permuted to q-block-major node order, so one
            # xbar DMA-transpose gives v^T with every q-block's 128 node
            # columns contiguous; kills the per-q-block PE transpose and
            # its PSUM->SBUF copy entirely.
            vt_all = cpool.tile([P, nc_nodes], f16)
            nc.scalar.dma_start_transpose(vt_all[:], vin[:])

            kd = k_nb - kp  # group-split: PE work can start once big_a lands

            def load_chunk(c):
                cs = slice(c * cw, (c + 1) * cw)
                big_a = kpool.tile([P, kp * cw], f8, tag="big_a")
                big_b = kpool.tile([P, kd * cw], f8, tag="big_b")
                eng = nc.scalar if (dual_ring and c % 2) else nc.sync
                eng.dma_start(
                    big_a[:].rearrange("p (k f) -> p k f", k=kp), nbr_r[:, :kp, cs]
                )
                eng.dma_start(
                    big_b[:].rearrange("p (k f) -> p k f", k=kd), nbr_r[:, kp:, cs]
                )

                def slab(k, lo, hi):
                    t = big_a if k < kp else big_b
                    return t[:, (k % kp) * cw + lo : (k % kp) * cw + hi]

                # DVE partial K-sum of the fp8 slabs, as a pair tree: the
                # first level reads 1-byte operands (1x mode); the upper
                # levels are fp16+fp16 in 2x mode.
                level = []
                ks = list(dve_ks)
                if len(ks) % 2:
                    t = spool.tile([P, cw], f16, tag="tp")
                    nc.vector.tensor_copy(out=t[:], in_=slab(ks[0], 0, cw))
                    level.append(t)
                    ks = ks[1:]
                for a, b2 in zip(ks[::2], ks[1::2]):
                    t = spool.tile([P, cw], f16, tag="tp")
                    nc.vector.tensor_add(
                        out=t[:], in0=slab(a, 0, cw), in1=slab(b2, 0, cw)
                    )
                    level.append(t)
                while len(level) > 1:
                    nxt = []
                    for i in range(0, len(level) - 1, 2):
                        t = spool.tile([P, cw], f16, tag="tp")
                        nc.vector.tensor_add(
                            out=t[:], in0=level[i][:], in1=level[i + 1][:]
                        )
                        nxt.append(t)
                    if len(level) % 2:
                        nxt.append(level[-1])
                    level = nxt
                return slab, level[0]

            # q-blocks are processed in groups of up to 4 sharing one PSUM
            # bank, so ACT does a few wide PSUM->SBUF copies per chunk
            # instead of one small copy per q-block.
            GRP = 4
            groups = [
                (g0, min(g0 + GRP, chunk_q)) for g0 in range(0, chunk_q, GRP)
            ]

            def finals(c, g0, g1, stb, ot):
                opb = pop.tile([P, GRP * O], f32, tag="OP")
                for qq in range(g0, g1):
                    off = (qq - g0) * O
                    gq = c * chunk_q + qq
                    os_ = slice(off, off + O)
                    nc.tensor.matmul(
                        opb[:, os_], lhsT=stb[:, off : off + P], rhs=wut_t[:],
                        start=True, stop=False,
                    )
                    nc.tensor.matmul(
                        opb[:, os_], lhsT=vt_all[:, gq * D : (gq + 1) * D],
                        rhs=wvtk_t[:], start=False, stop=False,
                    )
                    # bias as a rank-1 matmul: OP[n, o] += ones[n] * (K*b)[o]
                    nc.tensor.matmul(
                        opb[:, os_], lhsT=ones_t[:], rhs=bbc_t[:],
                        start=False, stop=True,
                    )
                nc.scalar.copy(ot[:, g0 * O : g1 * O], opb[:, : (g1 - g0) * O])

            def pe_chunk(c, slab, S):
                cs = slice(c * cw, (c + 1) * cw)
                ot = opool.tile([P, cw], f16, tag="ot")
                pending = None
                for g0, g1 in groups:
                    gw = (g1 - g0) * P
                    # S^T accumulation, one PSUM-bank column range per
                    # q-block: PE slabs, then the DVE partial, all as
                    # matmuls with identity moving.
                    # NOTE: each column range's accumulation group must stay
                    # contiguous on the PE queue — interleaving groups in one
                    # bank (even on disjoint columns) corrupts PSUM on HW.
                    STb = pst.tile([D, GRP * P], f32, tag="ST")
                    for qq in range(g0, g1):
                        off = (qq - g0) * P
                        ss = slice(off, off + P)
                        for j, k in enumerate(pe_ks):
                            nc.tensor.matmul(
                                STb[:, ss], lhsT=slab(k, qq * D, (qq + 1) * D),
                                rhs=iden8_t[:], start=(j == 0), stop=False,
                            )
                        nc.tensor.matmul(
                            STb[:, ss], lhsT=S[:, qq * D : (qq + 1) * D],
                            rhs=iden_t[:], start=False, stop=True,
                        )
                    stb = tpool.tile([D, GRP * P], f16, tag="st")
                    nc.scalar.copy(stb[:, :gw], STb[:, :gw])
                    if pending is not None:
                        finals(*pending)
                    pending = (c, g0, g1, stb, ot)
                finals(*pending)
                nc.scalar.dma_start(out_r[:, cs], ot[:])

            def repeat_body():
                if dma_only:
                    # Pure-DMA roofline probe: stream neighbors, copy one
                    # slab slice back out so DCE keeps the transfers.
                    for c in range(n_chunks):
                        cs = slice(c * cw, (c + 1) * cw)
                        big = kpool.tile([P, k_nb * cw], f8, tag="big")
                        eng = nc.scalar if (dual_ring and c % 2) else nc.sync
                        eng.dma_start(
                            big[:].rearrange("p (k f) -> p k f", k=k_nb),
                            nbr_r[:, :, cs],
                        )
                        nc.scalar.dma_start(
                            out_r[:, cs], big[:, 0 : 2 * cw].bitcast(f16)
                        )
                    return
                prev = None
                for c in range(n_chunks):
                    cur = (c, *load_chunk(c))
                    if prev is not None:
                        pe_chunk(*prev)
                    prev = cur
                pe_chunk(*prev)

            if loop_reps > 1:
                # Hardware loop: constant instruction count at any repeat
                # count, for noise-proof (t_hi - t_lo) timing.
                with tc.For_i(0, loop_reps, 1):
                    for _ in range(repeats):
                        repeat_body()
            else:
                for _ in range(repeats):
                    repeat_body()
    nc.compile()
    return nc


def _f8np():
    import concourse.mybir as mybir

    return mybir.dt.np(mybir.dt.float8e4)


def _prep_weights(W, b):
    W = np.asarray(W, dtype=np.float32)
    b = np.asarray(b, dtype=np.float32)
    Wv = W[:, :D]
    Wu = W[:, D:]
    wut = np.ascontiguousarray(Wu.T, dtype=np.float16)
    wvtk = np.ascontiguousarray((Wv.T * np.float32(K_NB)), dtype=np.float16)
    bbc = np.ascontiguousarray((np.float32(K_NB) * b).astype(np.float16))[None, :]
    ones = np.ones((1, P), dtype=np.float16)
    iden = np.eye(P, dtype=np.float16)
    iden8 = np.eye(P, dtype=_f8np())
    return wut, wvtk, bbc, ones, iden, iden8


def _make_in_maps(v, neighbors, W, b):
    wut, wvtk, bbc, ones, iden, iden8 = _prep_weights(W, b)
    v16 = np.asarray(v).astype(np.float16)
    n8 = np.asarray(neighbors).astype(_f8np())
    return [
        {
            "nbr": np.ascontiguousarray(n8[:, s : s + NC_NODES, :]),
            # q-block-major node order (node p*QB+q -> row q*P+p), so the
            # on-device xbar transpose yields contiguous q-block columns
            "vin": np.ascontiguousarray(
                v16[s : s + NC_NODES]
                .reshape(P, QB, D)
                .transpose(1, 0, 2)
                .reshape(NC_NODES, D)
            ),
            "wut": wut,
            "wvtk": wvtk,
            "bbc": bbc,
            "ones": ones,
            "iden": iden,
            "iden8": iden8,
        }
        for s in _core_starts()
    ]


def kernel(v, neighbors, W, b):
    from concourse.bass_utils import run_bass_kernel_spmd

    in_maps = _make_in_maps(v, neighbors, W, b)
    nc = _build()
    res = run_bass_kernel_spmd(nc, in_maps, core_ids=list(range(N_CORES)))

    out = np.empty((N_NODES, O), dtype=np.float32)
    step = N_NODES // N_CORES
    for c, s in enumerate(_core_starts()):
        own_lo = c * step
        own_hi = N_NODES if c == N_CORES - 1 else (c + 1) * step
        r = np.asarray(res.results[c]["out"], dtype=np.float32)
        out[own_lo:own_hi] = r[own_lo - s : own_hi - s]
    return out
